# revision 16
# baseline (speedup 1.0000x reference)
"""MoE routing kernel (nn_AMIR_11098195493476) for 8 TRN2 NeuronCores.

Strategy: data-parallel over tokens (N=32768 -> 4096/core), expert weights
replicated, no collectives. Per core:
  1. gating logits via PE matmul (logits^T [8, 4096]), PE-transpose to
     token-major, top-2 selection + softmax done as masked biases:
       combined = sum_e ghat_e * exp(y_e),  ghat = exp(l_e - l_max) for the
       top-2 (=> 1 and e^d), ~e^-30 for the rest; final output
       log(combined) - log(1 + e^d)  (the top-2 softmax normalizer).
  2. dense expert MLP in bf16: h = gelu(x @ w1[e] + b1), y = h @ w2[e] (+ b2
     via ones-row trick when nonzero), exp with per-token bias fused on ACT.
  3. accumulate over experts with a free-dim tensor_reduce, log, normalize.
"""

import sys
from contextlib import ExitStack

sys.path.insert(0, "/opt/trn_rl_repo")

import ml_dtypes
import numpy as np

import concourse.bass as bass
import concourse.tile as tile
from concourse import bacc, mybir
from concourse.bass_utils import run_bass_kernel_spmd

N, D, H, E, TOPK, NCORES = 32768, 192, 384, 8, 2, 8
TPC = N // NCORES          # tokens per core
NT = TPC // 128            # 128-token tiles per core
NCHUNK = TPC // 512        # 512-token chunks per core
F16 = mybir.dt.float16
F32 = mybir.dt.float32
AF = mybir.ActivationFunctionType
ALU = mybir.AluOpType

_GRAPH_CACHE = {}


def build_graph(use_b2: bool):
    nc = bacc.Bacc("TRN2", target_bir_lowering=False, debug=False)

    xembT = nc.dram_tensor("xembT", [4, 128, TPC], F16, kind="ExternalInput")
    xembTlo = nc.dram_tensor("xembTlo", [4, 128, TPC], F16, kind="ExternalInput")
    wgp = nc.dram_tensor("wgp", [4, 128, E], F16, kind="ExternalInput")
    wgplo = nc.dram_tensor("wgplo", [4, 128, E], F16, kind="ExternalInput")
    w1p = nc.dram_tensor("w1p", [2, 128, E * H], F16, kind="ExternalInput")
    w2p = nc.dram_tensor("w2p", [3, 128, E * D], F16, kind="ExternalInput")
    b2r = nc.dram_tensor("b2r", [1, E * D], F32, kind="ExternalInput")
    ident = nc.dram_tensor("ident", [8, 8], F32, kind="ExternalInput")
    outd = nc.dram_tensor("out", [TPC, D], F32, kind="ExternalOutput")

    with tile.TileContext(nc) as tc, ExitStack() as ctx:
        const = ctx.enter_context(tc.tile_pool(name="const", bufs=1))
        work = ctx.enter_context(tc.tile_pool(name="work", bufs=1))

        # ---- load everything ----
        xe = const.tile([128, 4, TPC], F16)
        for c in range(4):
            nc.sync.dma_start(xe[:, c, :], xembT[c])
        wg = const.tile([128, 4, E], F16)
        for c in range(4):
            nc.sync.dma_start(wg[:, c, :], wgp[c])
        wgl = const.tile([128, 4, E], F16)
        for c in range(4):
            nc.sync.dma_start(wgl[:, c, :], wgplo[c])
        xel = const.tile([128, 4, TPC], F16)
        for c in range(4):
            nc.sync.dma_start(xel[:, c, :], xembTlo[c])
        w1 = const.tile([128, 2, E * H], F16)
        for c in range(2):
            nc.sync.dma_start(w1[:, c, :], w1p[c])
        w2 = const.tile([128, 3, E * D], F16)
        for c in range(3):
            nc.sync.dma_start(w2[:, c, :], w2p[c])
        idt = const.tile([8, 8], F32)
        nc.sync.dma_start(idt[:], ident[:])
        if use_b2:
            b2s = const.tile([1, E * D], F32)
            nc.sync.dma_start(b2s[:], b2r[:])
            ones = const.tile([1, TPC], F16)
            nc.gpsimd.memset(ones[:], 1.0)

        # ---- gating: logitsT [8, TPC] ----
        with tc.tile_pool(name="gpsum", bufs=1, space="PSUM") as gpsum:
            lgT_ps = gpsum.tile([8, TPC], F32)
            for c in range(NCHUNK):
                # hi@wg_hi + hi@wg_lo + lo@wg_hi: ~f32-accurate logits so the
                # top-2 selection matches the f32 reference on near-ties.
                terms = [(wg, xe), (wgl, xe), (wg, xel)]
                for k in range(4):
                    for ti, (wgt, xet) in enumerate(terms):
                        nc.tensor.matmul(
                            lgT_ps[:, bass.ts(c, 512)],
                            wgt[:, k, :],
                            xet[:, k, bass.ts(c, 512)],
                            start=(k == 0 and ti == 0),
                            stop=(k == 3 and ti == 2),
                        )
            lgT = work.tile([8, TPC], F32)
            nc.vector.tensor_copy(lgT[:], lgT_ps[:])

        # transpose to token-major lg [128, NT, 8]
        with tc.tile_pool(name="tpsum", bufs=1, space="PSUM") as tpsum:
            lg_ps = tpsum.tile([128, NT, E], F32)
            for t in range(NT):
                nc.tensor.transpose(lg_ps[:, t, :], lgT[:, bass.ts(t, 128)], idt[:])
            lg = work.tile([128, NT, E], F32)
            nc.vector.tensor_copy(lg[:], lg_ps[:])

        # ---- routing (token-major) ----
        m1 = work.tile([128, NT], F32)
        m2 = work.tile([128, NT], F32)
        msk1 = work.tile([128, NT, E], F32)
        msk2 = work.tile([128, NT, E], F32)
        l2t = work.tile([128, NT, E], F32)
        for t in range(NT):
            nc.vector.tensor_reduce(
                m1[:, t : t + 1], lg[:, t, :], axis=mybir.AxisListType.X, op=ALU.max
            )
            nc.vector.tensor_scalar(
                msk1[:, t, :], lg[:, t, :], m1[:, t : t + 1], None,
                op0=ALU.is_equal,
            )
            nc.vector.scalar_tensor_tensor(
                l2t[:, t, :], msk1[:, t, :], -1e30, lg[:, t, :],
                op0=ALU.mult, op1=ALU.add,
            )
            nc.vector.tensor_reduce(
                m2[:, t : t + 1], l2t[:, t, :], axis=mybir.AxisListType.X, op=ALU.max
            )
            nc.vector.tensor_scalar(
                msk2[:, t, :], l2t[:, t, :], m2[:, t : t + 1], None,
                op0=ALU.is_equal,
            )
        # d = clamp(m2 - m1, -9.9, 0)
        dlt = work.tile([128, NT], F32)
        nc.vector.tensor_sub(dlt[:], m2[:], m1[:])
        nc.vector.tensor_scalar_max(dlt[:], dlt[:], -9.9)
        # biasT[t, e] = 0 if argmax, d if 2nd, -30 otherwise
        #             = (msk1+msk2)*30 - 30 + msk2*d
        biasT = work.tile([128, NT, E], F32)
        u = work.tile([128, NT, E], F32)
        nc.vector.tensor_add(u[:], msk1[:], msk2[:])
        nc.vector.tensor_scalar(u[:], u[:], 30.0, 30.0, op0=ALU.mult, op1=ALU.subtract)
        for t in range(NT):
            nc.vector.scalar_tensor_tensor(
                biasT[:, t, :], msk2[:, t, :], dlt[:, t : t + 1], u[:, t, :],
                op0=ALU.mult, op1=ALU.add,
            )

        # ---- expert MLP (dense, masked) ----
        acc = work.tile([128, NT, D], F32)
        with (
            tc.tile_pool(name="hps", bufs=2, space="PSUM") as hps,
            tc.tile_pool(name="yps", bufs=2, space="PSUM") as yps,
            tc.tile_pool(name="hsb", bufs=2) as hsb,
            tc.tile_pool(name="e3", bufs=1) as e3p,
        ):
            for c in range(NCHUNK):
                e3 = e3p.tile([128, 4, D, E], F32, tag="e3")
                for e in range(E):
                    h_ps = hps.tile([128, 3, 512], F32, tag="hps")
                    for mt in range(3):
                        for k in range(2):
                            nc.tensor.matmul(
                                h_ps[:, mt, :],
                                w1[:, k, e * H + mt * 128 : e * H + (mt + 1) * 128],
                                xe[:, k, bass.ts(c, 512)],
                                start=(k == 0),
                                stop=(k == 1),
                            )
                    h_sb = hsb.tile([128, 3, 512], F16, tag="hsb")
                    nc.scalar.activation(h_sb[:], h_ps[:], AF.Gelu)
                    for t in range(4):
                        tg = c * 4 + t
                        y_ps = yps.tile([128, D], F32, tag="yps")
                        for k2 in range(3):
                            nc.tensor.matmul(
                                y_ps[:],
                                h_sb[:, k2, bass.ts(t, 128)],
                                w2[:, k2, bass.ts(e, D)],
                                start=(k2 == 0),
                                stop=(k2 == 2) and not use_b2,
                            )
                        if use_b2:
                            nc.tensor.matmul(
                                y_ps[:],
                                ones[:, tg * 128 : tg * 128 + 128],
                                b2s[:, bass.ts(e, D)],
                                start=False,
                                stop=True,
                            )
                        nc.scalar.activation(
                            e3[:, t, :, e], y_ps[:], AF.Exp,
                            bias=biasT[:, tg, e : e + 1],
                        )
                for t in range(4):
                    tg = c * 4 + t
                    nc.vector.tensor_reduce(
                        acc[:, tg, :], e3[:, t, :, :],
                        axis=mybir.AxisListType.X, op=ALU.add,
                    )

        # ---- finalize: out = log(acc) - log(1 + e^d) ----
        ed = work.tile([128, NT], F32)
        nc.scalar.activation(ed[:], dlt[:], AF.Exp)
        nc.vector.tensor_scalar_add(ed[:], ed[:], 1.0)
        rcp = work.tile([128, NT], F32)
        nc.vector.reciprocal(rcp[:], ed[:])
        for t in range(NT):
            nc.vector.tensor_scalar_mul(acc[:, t, :], acc[:, t, :], rcp[:, t : t + 1])
        outsb = work.tile([128, NT, D], F32)
        nc.scalar.activation(outsb[:], acc[:], AF.Ln)
        nc.sync.dma_start(outd.rearrange("(g p) d -> p g d", p=128), outsb[:])

    nc.compile()
    return nc


def _prep_inputs(x, emb, w_gate, w1, b1, w2, b2):
    """Build per-core input maps (host-side sharding + layout prep only)."""
    bf = np.float16
    xT = np.ascontiguousarray(x.T)      # [D, N]
    embT = np.ascontiguousarray(emb.T)  # [D, N]

    wgp32 = np.zeros([4, 128, E], dtype=np.float32)
    wgp32[0] = w_gate[0:128]
    wgp32[1, 0:64] = w_gate[128:192]
    wgp32[2] = w_gate[192:320]
    wgp32[3, 0:64] = w_gate[320:384]
    wgp = wgp32.astype(bf)
    wgplo = (wgp32 - wgp.astype(np.float32)).astype(bf)

    w1p = np.zeros([2, 128, E, H], dtype=bf)
    w1p[0] = np.transpose(w1[:, 0:128, :], (1, 0, 2))
    w1p[1, 0:64] = np.transpose(w1[:, 128:192, :], (1, 0, 2))
    w1p[1, 64] = b1  # pairs with the ones-row in xembT chunk 1
    w1p = w1p.reshape(2, 128, E * H)

    w2p = np.transpose(w2, (1, 0, 2)).reshape(3, 128, E, D)
    w2p = np.ascontiguousarray(w2p).astype(bf).reshape(3, 128, E * D)

    b2r = np.ascontiguousarray(b2.astype(np.float32).reshape(1, E * D))
    ident = np.eye(8, dtype=np.float32)

    in_maps = []
    for i in range(NCORES):
        s = slice(i * TPC, (i + 1) * TPC)
        xembT32 = np.zeros([4, 128, TPC], dtype=np.float32)
        xembT32[0] = xT[0:128, s]
        xembT32[1, 0:64] = xT[128:192, s]
        xembT32[1, 64] = 1.0  # ones row -> b1 via w1p[1,64]
        xembT32[2] = embT[0:128, s]
        xembT32[3, 0:64] = embT[128:192, s]
        xembT = xembT32.astype(bf)
        xembTlo = (xembT32 - xembT.astype(np.float32)).astype(bf)
        in_maps.append(
            {
                "xembT": xembT,
                "xembTlo": xembTlo,
                "wgp": wgp,
                "wgplo": wgplo,
                "w1p": w1p,
                "w2p": w2p,
                "b2r": b2r,
                "ident": ident,
            }
        )
    return in_maps


def timed_run(nc, in_maps, reps, iters=6):
    """Measure per-execution HW time by chaining `reps` NEFF executions in
    one jitted dispatch (output buffer threaded through as the next exec's
    donated-zero input, forcing serialization). Tunnel/dispatch overhead
    cancels in the difference between reps and 1."""
    import time as _time

    import jax
    from jax.sharding import Mesh, PartitionSpec

    try:
        from jax.experimental.shard_map import shard_map
    except ImportError:
        from jax.shard_map import shard_map

    from concourse import bass2jax as b2j
    from concourse import mybir as mb

    b2j.install_neuronx_cc_hook()
    pname = nc.partition_id_tensor.name if nc.partition_id_tensor else None
    in_names, out_names, out_avals, zero_outs = [], [], [], []
    for alloc in nc.m.functions[0].allocations:
        if not isinstance(alloc, mb.MemoryLocationSet):
            continue
        name = alloc.memorylocations[0].name
        if alloc.kind == "ExternalInput":
            if name != pname:
                in_names.append(name)
        elif alloc.kind == "ExternalOutput":
            shape = tuple(alloc.tensor_shape)
            dt = mb.dt.np(alloc.dtype)
            out_names.append(name)
            out_avals.append(jax.core.ShapedArray(shape, dt))
            zero_outs.append(np.zeros(shape, dt))
    n_params = len(in_names)
    in_names.extend(out_names)
    if pname is not None:
        in_names.append(pname)

    def make_body():
        def _body(*args):
            ops = list(args)
            if pname is not None:
                ops.append(b2j.partition_id_tensor())
            return tuple(
                b2j._bass_exec_p.bind(
                    *ops,
                    out_avals=tuple(out_avals),
                    in_names=tuple(in_names),
                    out_names=tuple(out_names),
                    lowering_input_output_aliases=(),
                    sim_require_finite=True,
                    sim_require_nnan=True,
                    nc=nc,
                )
            )

        return _body

    ncores = len(in_maps)
    devices = jax.devices()[:ncores]
    mesh = Mesh(np.asarray(devices), ("core",))
    in_specs = (PartitionSpec("core"),) * (n_params + len(out_names))
    out_specs = (PartitionSpec("core"),) * len(out_names)
    concat_in = [
        np.concatenate([np.asarray(in_maps[c][nm]) for c in range(ncores)], axis=0)
        for nm in in_names[:n_params]
    ]
    concat_zeros = [
        np.zeros((ncores * z.shape[0], *z.shape[1:]), z.dtype) for z in zero_outs
    ]

    fn = jax.jit(
        shard_map(
            make_body(), mesh=mesh, in_specs=in_specs, out_specs=out_specs,
            check_rep=False,
        ),
        keep_unused=True,
    )
    from jax.sharding import NamedSharding

    sh = NamedSharding(mesh, PartitionSpec("core"))
    din = [jax.device_put(a, sh) for a in concat_in]
    dzero = [jax.device_put(z, sh) for z in concat_zeros]
    jax.block_until_ready(fn(*din, *dzero))  # compile + warm
    samples = []
    for _ in range(iters):
        t0 = _time.perf_counter()
        jax.block_until_ready(fn(*din, *dzero))
        samples.append(_time.perf_counter() - t0)
    return sorted(samples)


def build_trivial():
    """Minimal NEFF used to measure the PJRT/tunnel dispatch floor."""
    nc = bacc.Bacc("TRN2", target_bir_lowering=False, debug=False)
    a = nc.dram_tensor("a", [128, 8], F16, kind="ExternalInput")
    o = nc.dram_tensor("out", [128, 8], F16, kind="ExternalOutput")
    with tile.TileContext(nc) as tc, ExitStack() as ctx:
        pool = ctx.enter_context(tc.tile_pool(name="p", bufs=1))
        t = pool.tile([128, 8], F16)
        nc.sync.dma_start(t[:], a[:])
        nc.sync.dma_start(o[:], t[:])
    nc.compile()
    return nc


def kernel(x, emb, w_gate, w1, b1, w2, b2, _trace=False):
    x = np.asarray(x, dtype=np.float32)
    emb = np.asarray(emb, dtype=np.float32)
    w_gate = np.asarray(w_gate, dtype=np.float32)
    w1 = np.asarray(w1, dtype=np.float32)
    b1 = np.asarray(b1, dtype=np.float32)
    w2 = np.asarray(w2, dtype=np.float32)
    b2 = np.asarray(b2, dtype=np.float32)

    use_b2 = bool(np.any(b2))
    key = ("dense", use_b2)
    if key not in _GRAPH_CACHE:
        _GRAPH_CACHE[key] = build_graph(use_b2)
    nc = _GRAPH_CACHE[key]

    in_maps = _prep_inputs(x, emb, w_gate, w1, b1, w2, b2)
    res = run_bass_kernel_spmd(nc, in_maps, core_ids=list(range(NCORES)))
    out = np.concatenate([r["out"] for r in res.results], axis=0)
    return out


if __name__ == "__main__":
    rng = np.random.default_rng(0)
    ins = {
        "x": rng.standard_normal((N, D), dtype=np.float32),
        "emb": rng.standard_normal((N, D), dtype=np.float32),
        "w_gate": rng.standard_normal((2 * D, E), dtype=np.float32) * 0.05,
        "w1": rng.standard_normal((E, D, H), dtype=np.float32) * 0.02,
        "b1": np.zeros((E, H), np.float32),
        "w2": rng.standard_normal((E, H, D), dtype=np.float32) * 0.02,
        "b2": np.zeros((E, D), np.float32),
    }
    out = kernel(**ins)
    print("out", out.shape, out.dtype, np.abs(out).mean())
